# revision 1
# baseline (speedup 1.0000x reference)
"""Cross-attention Trainium2 kernel for nn_CrossAttention_37495064494692.

B=8 batches sharded 1/core across 8 NeuronCores (data parallel).
Per core: full cross-attention for one batch element, computed in
feature-on-partitions ("transposed") layouts so no on-device transposes
are needed. Matmuls run in bf16 (fp32 PSUM accumulation); biases and
softmax math stay fp32.

  gT  = Wt^T @ guideT + bt        [E, L]
  KT  = Wk^T @ gT + bk            [E, L]   (head h = rows h*64..h*64+64)
  V   = gT^T @ Wv + bv            [L, E]   stored padded with a ones
                                           column per head (v_aug) so the
                                           PV matmul also produces the
                                           softmax denominator row.
  QT  = Wq^T @ queryT + bq        [E, S]   (streamed per 512-col s-chunk)
  sT  = kT_h^T(L-tile) @ qT_h     [L, S]   K=64, two heads packed into
                                           the PE array at row 0 / 64
  aT  = exp(SCALE*sT + maskbias)  [L, S]
  OT_h = [v_h | 1]^T @ aT         [65, S]  row 64 = sum_l aT = denom
  OT   = OT_h / denom             (batched reciprocal + partition
                                   broadcast + mul)
  out  = OT^T @ Wo + bo           [S, E]   natural layout, DMA'd out
"""
import sys

sys.path.insert(0, "/opt/trn_rl_repo")

import ml_dtypes
import numpy as np

import concourse.bacc as bacc
import concourse.bass as bass
import concourse.tile as tile
from concourse import mybir
from concourse.bass_utils import run_bass_kernel_spmd

F32 = mybir.dt.float32
BF16 = mybir.dt.bfloat16
MMDT = BF16                      # dtype of all matmul operands
NPDT = ml_dtypes.bfloat16        # matching numpy dtype for host inputs

B, S, L = 8, 2048, 512
E, TE, H = 1024, 768, 16
D = E // H
SCALE = D ** -0.5

SC = 512              # s-chunk width
N_SC = S // SC        # 4 s-chunks
N_E = E // 128        # 8 E-chunks
N_TE = TE // 128      # 6 TE-chunks
N_LT = L // 128       # 4 L-tiles
HP = H // 2           # 8 head pairs

TRACE = False
_CACHED_NC = None


def build_nc():
    nc = bacc.Bacc()

    queryT = nc.declare_dram_parameter("queryT", [E, S], MMDT, isOutput=False)
    guideT = nc.declare_dram_parameter("guideT", [TE, L], MMDT, isOutput=False)
    Wt = nc.declare_dram_parameter("Wt", [TE, E], MMDT, isOutput=False)
    Wq = nc.declare_dram_parameter("Wq", [E, E], MMDT, isOutput=False)
    Wkv = nc.declare_dram_parameter("Wkv", [E, 2 * E], MMDT, isOutput=False)
    Wo = nc.declare_dram_parameter("Wo", [E, E], MMDT, isOutput=False)
    bt = nc.declare_dram_parameter("bt", [E], F32, isOutput=False)
    bq = nc.declare_dram_parameter("bq", [E], F32, isOutput=False)
    bk = nc.declare_dram_parameter("bk", [E], F32, isOutput=False)
    bv_r = nc.declare_dram_parameter("bv_r", [E], MMDT, isOutput=False)
    bo = nc.declare_dram_parameter("bo", [E], F32, isOutput=False)
    mbias = nc.declare_dram_parameter("mbias", [L], F32, isOutput=False)
    out = nc.declare_dram_parameter("out", [S, E], F32, isOutput=True)

    Exp = mybir.ActivationFunctionType.Exp

    with tile.TileContext(nc) as tc:
        with (
            tc.tile_pool(name="res", bufs=1) as res,
            tc.tile_pool(name="psA", bufs=3, space="PSUM") as psA,
            tc.tile_pool(name="psS", bufs=3, space="PSUM") as psS,
            tc.tile_pool(name="psV", bufs=2, space="PSUM") as psV,
        ):
            # ---- resident small tensors ----
            bt_sb = res.tile([128, N_E], F32, tag="bt")
            bq_sb = res.tile([128, N_E], F32, tag="bq")
            bk_sb = res.tile([128, N_E], F32, tag="bk")
            mb_sb = res.tile([128, N_LT], F32, tag="mb")
            nc.sync.dma_start(out=bt_sb, in_=bt.rearrange("(t p) -> p t", p=128))
            nc.sync.dma_start(out=bq_sb, in_=bq.rearrange("(t p) -> p t", p=128))
            nc.sync.dma_start(out=bk_sb, in_=bk.rearrange("(t p) -> p t", p=128))
            nc.sync.dma_start(out=mb_sb, in_=mbias.rearrange("(t p) -> p t", p=128))
            bo_bc = res.tile([128, E], F32, tag="bo")
            bo_ap = bo[:]
            nc.gpsimd.dma_start(
                out=bo_bc,
                in_=bass.AP(tensor=bo_ap.tensor, offset=bo_ap.offset,
                            ap=[[0, 128], [1, E]]),
            )
            bv_row = res.tile([1, E], MMDT, tag="bvr")
            nc.sync.dma_start(out=bv_row, in_=bv_r.rearrange("(one f) -> one f", one=1))
            ones_f = res.tile([1, 128], F32, tag="ones_f")
            ones_r = res.tile([1, 128], MMDT, tag="ones_r")
            nc.vector.memset(ones_f, 1.0)
            nc.scalar.copy(ones_r, ones_f)
            onesc_f = res.tile([128, H], F32, tag="onesc")
            nc.vector.memset(onesc_f, 1.0)

            # ---- long-lived activations ----
            KT = [res.tile([128, L], MMDT, tag=f"KT{j}", name=f"KT{j}")
                  for j in range(N_E)]
            # v_aug: head h occupies cols h*65..h*65+64, col h*65+64 == 1.0
            Vt = [res.tile([128, H * (D + 1)], MMDT, tag=f"V{lt}", name=f"V{lt}")
                  for lt in range(N_LT)]

            # ================= prologue =================
            with tc.tile_pool(name="pro", bufs=1) as pro:
                g_in = [pro.tile([128, L], MMDT, tag=f"gin{t}", name=f"gin{t}")
                        for t in range(N_TE)]
                for t in range(N_TE):
                    nc.sync.dma_start(out=g_in[t], in_=guideT[t * 128:(t + 1) * 128, :])
                Wt_sb = [pro.tile([128, E], MMDT, tag=f"wt{t}", name=f"wt{t}")
                         for t in range(N_TE)]
                for t in range(N_TE):
                    nc.sync.dma_start(out=Wt_sb[t], in_=Wt[t * 128:(t + 1) * 128, :])

                gT = [pro.tile([128, L], MMDT, tag=f"gT{j}", name=f"gT{j}")
                      for j in range(N_E)]
                for j in range(N_E):
                    ps = psA.tile([128, SC], F32, tag="acc")
                    for t in range(N_TE):
                        nc.tensor.matmul(
                            ps, lhsT=Wt_sb[t][:, j * 128:(j + 1) * 128], rhs=g_in[t],
                            start=(t == 0), stop=(t == N_TE - 1),
                        )
                    nc.vector.tensor_scalar_add(gT[j], ps, bt_sb[:, j:j + 1])

                Wkv_sb = [pro.tile([128, 2 * E], MMDT, tag=f"wkv{e}", name=f"wkv{e}")
                          for e in range(N_E)]
                for e in range(N_E):
                    nc.sync.dma_start(out=Wkv_sb[e], in_=Wkv[e * 128:(e + 1) * 128, :])

                # KT = Wk^T @ gT + bk
                for j in range(N_E):
                    ps = psA.tile([128, SC], F32, tag="acc")
                    for e in range(N_E):
                        nc.tensor.matmul(
                            ps, lhsT=Wkv_sb[e][:, j * 128:(j + 1) * 128], rhs=gT[e],
                            start=(e == 0), stop=(e == N_E - 1),
                        )
                    nc.vector.tensor_scalar_add(KT[j], ps, bk_sb[:, j:j + 1])

                # V = gT^T @ Wv + bv, written into v_aug layout
                for lt in range(N_LT):
                    va = Vt[lt].rearrange("p (h c) -> p h c", c=D + 1)
                    for half in range(2):
                        ps = psA.tile([128, SC], F32, tag="acc")
                        for e in range(N_E):
                            nc.tensor.matmul(
                                ps, lhsT=gT[e][:, lt * 128:(lt + 1) * 128],
                                rhs=Wkv_sb[e][:, E + half * SC: E + (half + 1) * SC],
                                start=(e == 0), stop=False,
                            )
                        nc.tensor.matmul(
                            ps, lhsT=ones_r,
                            rhs=bv_row[:, half * SC:(half + 1) * SC],
                            start=False, stop=True,
                        )
                        nc.vector.tensor_copy(
                            va[:, half * 8:(half + 1) * 8, 0:D],
                            ps.rearrange("p (h c) -> p h c", c=D),
                        )
                    nc.vector.tensor_copy(
                        va[:, :, D:D + 1],
                        onesc_f.rearrange("p (h c) -> p h c", c=1),
                    )

            # ================= main loop =================
            with tc.tile_pool(name="mn", bufs=1) as mn, \
                 tc.tile_pool(name="io", bufs=2) as io, \
                 tc.tile_pool(name="st", bufs=3) as stp, \
                 tc.tile_pool(name="dnsp", bufs=2) as dnsp, \
                 tc.tile_pool(name="dr", bufs=2, space="DRAM") as drp:
                Wq_sb = [mn.tile([128, E], MMDT, tag=f"wq{e}", name=f"wq{e}")
                         for e in range(N_E)]
                for e in range(N_E):
                    nc.sync.dma_start(out=Wq_sb[e], in_=Wq[e * 128:(e + 1) * 128, :])
                Wo_sb = [mn.tile([128, E], MMDT, tag=f"wo{e}", name=f"wo{e}")
                         for e in range(N_E)]
                for e in range(N_E):
                    nc.sync.dma_start(out=Wo_sb[e], in_=Wo[e * 128:(e + 1) * 128, :])

                for c in range(N_SC):
                    scs = slice(c * SC, (c + 1) * SC)
                    qT_in = [io.tile([128, SC], MMDT, tag=f"qin{e}", name=f"qin{e}_{c}")
                             for e in range(N_E)]
                    for e in range(N_E):
                        nc.sync.dma_start(out=qT_in[e],
                                          in_=queryT[e * 128:(e + 1) * 128, scs])

                    QT = [io.tile([128, SC], MMDT, tag=f"QT{j}", name=f"QT{j}_{c}")
                          for j in range(N_E)]
                    for j in range(N_E):
                        ps = psA.tile([128, SC], F32, tag="acc")
                        for e in range(N_E):
                            nc.tensor.matmul(
                                ps, lhsT=Wq_sb[e][:, j * 128:(j + 1) * 128],
                                rhs=qT_in[e],
                                start=(e == 0), stop=(e == N_E - 1),
                            )
                        nc.vector.tensor_scalar_add(QT[j], ps, bq_sb[:, j:j + 1])

                    OT = [io.tile([128, SC], MMDT, tag=f"OT{j}", name=f"OT{j}_{c}")
                          for j in range(N_E)]
                    dns = dnsp.tile([97, 4 * SC], F32, tag="dns", name=f"dns_{c}")
                    denom = stp.tile([H, SC], F32, tag="dn", name=f"dn_{c}")
                    recips = stp.tile([H, SC], F32, tag="rc", name=f"rc_{c}")
                    for hp in range(HP):
                        att = [[io.tile([128, SC], MMDT, tag=f"at{u}{lt}",
                                        name=f"at{u}{lt}_{c}_{hp}")
                                for lt in range(N_LT)] for u in range(2)]
                        for lt in range(N_LT):
                            lts = slice(lt * 128, (lt + 1) * 128)
                            for u in range(2):
                                rows = slice(u * 64, (u + 1) * 64)
                                ps = psS.tile([128, SC], F32, tag="sc")
                                nc.tensor.matmul(
                                    ps, lhsT=KT[hp][rows, lts], rhs=QT[hp][rows, :],
                                    start=True, stop=True,
                                )
                                nc.scalar.activation(
                                    att[u][lt], ps, Exp,
                                    bias=mb_sb[:, lt:lt + 1], scale=SCALE,
                                )
                        for u in range(2):
                            h = 2 * hp + u
                            pv = psV.tile([D + 1, SC], F32, tag="pv",
                                          name=f"pv{h}_{c}")
                            for lt in range(N_LT):
                                nc.tensor.matmul(
                                    pv, lhsT=Vt[lt][:, h * (D + 1):(h + 1) * (D + 1)],
                                    rhs=att[u][lt],
                                    start=(lt == 0), stop=(lt == N_LT - 1),
                                )
                            # stash denom row in a 32-aligned slot + raw output
                            nc.vector.tensor_copy(
                                dns[32 * (h // 4):32 * (h // 4) + 1,
                                    (h % 4) * SC:(h % 4 + 1) * SC],
                                pv[D:D + 1, :],
                            )
                            nc.vector.tensor_copy(
                                OT[hp][u * 64:(u + 1) * 64, :], pv[0:D, :])
                    # gather the 16 denom rows onto 16 partitions, one recip
                    for h in range(H):
                        nc.sync.dma_start(
                            out=denom[h:h + 1, :],
                            in_=dns[32 * (h // 4):32 * (h // 4) + 1,
                                    (h % 4) * SC:(h % 4 + 1) * SC],
                        )
                    nc.vector.reciprocal(recips, denom)
                    rc_d = drp.tile([H, SC], F32, tag="rcd", name=f"rcd_{c}")
                    nc.gpsimd.dma_start(out=rc_d, in_=recips)
                    for h in range(H):
                        bc = stp.tile([128, SC], F32, tag="bc")
                        rsl = rc_d[h:h + 1, :]
                        nc.gpsimd.dma_start(
                            out=bc,
                            in_=bass.AP(tensor=rsl.tensor, offset=rsl.offset,
                                        ap=[[0, 128], [1, SC]]),
                        )
                        rows = slice((h % 2) * 64, (h % 2 + 1) * 64)
                        nc.vector.tensor_mul(
                            OT[h // 2][rows, :], OT[h // 2][rows, :], bc[rows, :])

                    # out = OT^T @ Wo + bo  (natural layout)
                    for st in range(N_LT):
                        sts = slice(st * 128, (st + 1) * 128)
                        for half in range(2):
                            ps = psA.tile([128, SC], F32, tag="acc")
                            for j in range(N_E):
                                nc.tensor.matmul(
                                    ps, lhsT=OT[j][:, sts],
                                    rhs=Wo_sb[j][:, half * SC:(half + 1) * SC],
                                    start=(j == 0), stop=(j == N_E - 1),
                                )
                            ob = stp.tile([128, SC], F32, tag="ob")
                            nc.vector.tensor_add(
                                ob, ps, bo_bc[:, half * SC:(half + 1) * SC])
                            nc.sync.dma_start(
                                out=out[c * SC + st * 128: c * SC + (st + 1) * 128,
                                        half * SC:(half + 1) * SC],
                                in_=ob,
                            )

    if not nc.is_finalized():
        nc.finalize()
    return nc


def kernel(query, guide_vector, attention_mask, Wt, bt, Wq, bq, Wkv, bkv, Wo, bo):
    global _CACHED_NC
    query = np.asarray(query, dtype=np.float32)
    guide_vector = np.asarray(guide_vector, dtype=np.float32)
    attention_mask = np.asarray(attention_mask)
    bt = np.asarray(bt, dtype=np.float32)
    bq = np.asarray(bq, dtype=np.float32)
    bkv = np.asarray(bkv, dtype=np.float32)
    bo = np.asarray(bo, dtype=np.float32)
    Wt_m = np.asarray(Wt, dtype=np.float32).astype(NPDT)
    Wq_m = np.asarray(Wq, dtype=np.float32).astype(NPDT)
    Wkv_m = np.asarray(Wkv, dtype=np.float32).astype(NPDT)
    Wo_m = np.asarray(Wo, dtype=np.float32).astype(NPDT)
    bv_m = bkv[E:].astype(NPDT)

    if _CACHED_NC is None:
        _CACHED_NC = build_nc()
    nc = _CACHED_NC

    mb = np.where(attention_mask == 0, np.float32(-1e9), np.float32(0.0))
    in_maps = []
    for b in range(B):
        in_maps.append({
            "queryT": np.ascontiguousarray(query[b].T).astype(NPDT),
            "guideT": np.ascontiguousarray(guide_vector[b].T).astype(NPDT),
            "Wt": Wt_m, "Wq": Wq_m, "Wkv": Wkv_m, "Wo": Wo_m,
            "bt": bt, "bq": bq, "bk": bkv[:E], "bv_r": bv_m,
            "bo": bo, "mbias": mb[b].astype(np.float32),
        })
    res = run_bass_kernel_spmd(nc, in_maps, list(range(B)), trace=TRACE)
    if TRACE:
        kernel.last_exec_time_ns = res.exec_time_ns
        kernel.last_results = res
    return np.stack([res.results[b]["out"] for b in range(B)])



# revision 3
# speedup vs baseline: 1.3683x; 1.3683x over previous
"""Cross-attention Trainium2 kernel for nn_CrossAttention_37495064494692.

B=8 batches sharded 1/core across 8 NeuronCores (data parallel).
Per core: full cross-attention for one batch element in
feature-on-partitions ("transposed") layouts. Matmuls in bf16 with fp32
PSUM accumulation; softmax math fp32.

Host-side the text projection is folded into the kv projection:
  Wk = Wt @ Wkv[:, :E],  bk = bt @ Wkv[:, :E] + bkv[:E]   (same for v)
so the device computes
  KT  = Wk^T @ guideT + bk          [E, L]
  V   = guideT^T @ Wv + bv          [L, E]  stored per head with a ones
                                    column (v_aug) so the PV matmul also
                                    emits the softmax denominator row
  QT  = Wq^T @ queryT + bq          [E, S]  per 512-col s-chunk
  sT  = KT_h^T(l-tile) @ QT_h       [L, S]  two heads -> one 2-bank PSUM
                                    pair, one [128,1024] EXP each
  OTr_h = [v_h | 1]^T @ aT          [65, S] row 64 = denominator
  OT_h = OTr_h * bcast(1/denom)     bcast via K=1 matmul (no DRAM trip)
  out  = OT^T @ Wo + bo             [S, E]

The main loop software-pipelines across s-chunks: QT(c+1) and
out-projection(c-1) matmul chains are interleaved ("pumped") between the
score/PV bursts of chunk c so the PE never idles while the ACT engine
works through the EXPs.
"""
import sys

sys.path.insert(0, "/opt/trn_rl_repo")

from collections import deque

import ml_dtypes
import numpy as np

import concourse.bacc as bacc
import concourse.bass as bass
import concourse.tile as tile
from concourse import mybir
from concourse.bass_utils import run_bass_kernel_spmd

F32 = mybir.dt.float32
BF16 = mybir.dt.bfloat16
MMDT = BF16
NPDT = ml_dtypes.bfloat16

B, S, L = 8, 2048, 512
E, TE, H = 1024, 768, 16
D = E // H
SCALE = D ** -0.5

SC = 512              # s-chunk width
N_SC = S // SC        # 4 s-chunks
N_E = E // 128        # 8 E-chunks
N_TE = TE // 128      # 6 TE-chunks
N_LT = L // 128       # 4 L-tiles
HP = H // 2           # 8 head pairs

TRACE = False
_CACHED_NC = None


def build_nc():
    nc = bacc.Bacc()

    queryT = nc.declare_dram_parameter("queryT", [E, S], MMDT, isOutput=False)
    guideT = nc.declare_dram_parameter("guideT", [TE, L], MMDT, isOutput=False)
    Wq = nc.declare_dram_parameter("Wq", [E, E], MMDT, isOutput=False)
    Wk = nc.declare_dram_parameter("Wk", [TE, E], MMDT, isOutput=False)
    Wv = nc.declare_dram_parameter("Wv", [TE, E], MMDT, isOutput=False)
    Wo = nc.declare_dram_parameter("Wo", [E, E], MMDT, isOutput=False)
    bq = nc.declare_dram_parameter("bq", [E], F32, isOutput=False)
    bk = nc.declare_dram_parameter("bk", [E], F32, isOutput=False)
    bv_r = nc.declare_dram_parameter("bv_r", [E], MMDT, isOutput=False)
    bo = nc.declare_dram_parameter("bo", [E], F32, isOutput=False)
    mbias = nc.declare_dram_parameter("mbias", [L], F32, isOutput=False)
    out = nc.declare_dram_parameter("out", [S, E], F32, isOutput=True)

    Exp = mybir.ActivationFunctionType.Exp

    with tile.TileContext(nc) as tc:
        with (
            tc.tile_pool(name="res", bufs=1) as res,
            tc.tile_pool(name="io", bufs=2) as io,
            tc.tile_pool(name="stp", bufs=3) as stp,
            tc.tile_pool(name="psA", bufs=2, space="PSUM") as psA,
            tc.tile_pool(name="psS", bufs=2, space="PSUM") as psS,
            tc.tile_pool(name="psV", bufs=2, space="PSUM") as psV,
        ):
            # ---- resident small tensors ----
            bq_sb = res.tile([128, N_E], F32, tag="bq")
            bk_sb = res.tile([128, N_E], F32, tag="bk")
            mb_sb = res.tile([128, N_LT], F32, tag="mb")
            nc.sync.dma_start(out=bq_sb, in_=bq.rearrange("(t p) -> p t", p=128))
            nc.sync.dma_start(out=bk_sb, in_=bk.rearrange("(t p) -> p t", p=128))
            nc.sync.dma_start(out=mb_sb, in_=mbias.rearrange("(t p) -> p t", p=128))
            bo_bc = res.tile([128, E], F32, tag="bo")
            bo_ap = bo[:]
            nc.gpsimd.dma_start(
                out=bo_bc,
                in_=bass.AP(tensor=bo_ap.tensor, offset=bo_ap.offset,
                            ap=[[0, 128], [1, E]]),
            )
            bv_row = res.tile([1, E], MMDT, tag="bvr")
            nc.sync.dma_start(out=bv_row, in_=bv_r.rearrange("(one f) -> one f", one=1))
            ones_f = res.tile([1, 128], F32, tag="ones_f")
            ones_r = res.tile([1, 128], MMDT, tag="ones_r")
            nc.vector.memset(ones_f, 1.0)
            nc.scalar.copy(ones_r, ones_f)
            onesc_f = res.tile([128, H], F32, tag="onesc")
            nc.vector.memset(onesc_f, 1.0)

            # ---- long-lived activations + resident weights ----
            KT = [res.tile([128, L], MMDT, tag=f"KT{j}", name=f"KT{j}")
                  for j in range(N_E)]
            # v_aug: head h cols h*65..h*65+64, col h*65+64 == 1.0
            Vt = [res.tile([128, H * (D + 1)], MMDT, tag=f"V{lt}", name=f"V{lt}")
                  for lt in range(N_LT)]
            Wq_sb = [res.tile([128, E], MMDT, tag=f"wq{e}", name=f"wq{e}")
                     for e in range(N_E)]
            Wo_sb = [res.tile([128, E], MMDT, tag=f"wo{e}", name=f"wo{e}")
                     for e in range(N_E)]

            # chunk-0 query + weights prefetch on the gpsimd DMA queue
            qin0 = [io.tile([128, SC], MMDT, tag=f"qin{e}", name=f"qin{e}_0")
                    for e in range(N_E)]
            for e in range(N_E):
                nc.gpsimd.dma_start(out=qin0[e], in_=queryT[e * 128:(e + 1) * 128, 0:SC])
            for e in range(N_E):
                nc.gpsimd.dma_start(out=Wq_sb[e], in_=Wq[e * 128:(e + 1) * 128, :])
            for e in range(N_E):
                nc.gpsimd.dma_start(out=Wo_sb[e], in_=Wo[e * 128:(e + 1) * 128, :])

            # ================= prologue: KT and V =================
            with tc.tile_pool(name="pro", bufs=1) as pro:
                g_in = [pro.tile([128, L], MMDT, tag=f"gin{t}", name=f"gin{t}")
                        for t in range(N_TE)]
                Wk_sb = [pro.tile([128, E], MMDT, tag=f"wk{t}", name=f"wk{t}")
                         for t in range(N_TE)]
                Wv_sb = [pro.tile([128, E], MMDT, tag=f"wv{t}", name=f"wv{t}")
                         for t in range(N_TE)]
                for t in range(N_TE):
                    nc.sync.dma_start(out=g_in[t], in_=guideT[t * 128:(t + 1) * 128, :])
                for t in range(N_TE):
                    nc.sync.dma_start(out=Wk_sb[t], in_=Wk[t * 128:(t + 1) * 128, :])
                for t in range(N_TE):
                    nc.sync.dma_start(out=Wv_sb[t], in_=Wv[t * 128:(t + 1) * 128, :])

                # KT = Wk^T @ guideT + bk
                for j in range(N_E):
                    ps = psA.tile([128, SC], F32, tag="acc", name=f"kacc{j}")
                    for t in range(N_TE):
                        nc.tensor.matmul(
                            ps, lhsT=Wk_sb[t][:, j * 128:(j + 1) * 128], rhs=g_in[t],
                            start=(t == 0), stop=(t == N_TE - 1),
                        )
                    nc.vector.tensor_scalar_add(KT[j], ps, bk_sb[:, j:j + 1])

                # V = guideT^T @ Wv + bv, into v_aug layout
                for lt in range(N_LT):
                    va = Vt[lt].rearrange("p (h c) -> p h c", c=D + 1)
                    for half in range(2):
                        ps = psA.tile([128, SC], F32, tag="acc", name=f"vacc{lt}_{half}")
                        for t in range(N_TE):
                            nc.tensor.matmul(
                                ps, lhsT=g_in[t][:, lt * 128:(lt + 1) * 128],
                                rhs=Wv_sb[t][:, half * SC:(half + 1) * SC],
                                start=(t == 0), stop=False,
                            )
                        nc.tensor.matmul(
                            ps, lhsT=ones_r,
                            rhs=bv_row[:, half * SC:(half + 1) * SC],
                            start=False, stop=True,
                        )
                        nc.vector.tensor_copy(
                            va[:, half * 8:(half + 1) * 8, 0:D],
                            ps.rearrange("p (h c) -> p h c", c=D),
                        )
                    nc.vector.tensor_copy(
                        va[:, :, D:D + 1],
                        onesc_f.rearrange("p (h c) -> p h c", c=1),
                    )

            # ================= main loop =================
            fillers = deque()

            def pump(n):
                while n > 0 and fillers:
                    try:
                        next(fillers[0])
                        n -= 1
                    except StopIteration:
                        fillers.popleft()

            def drain():
                while fillers:
                    try:
                        next(fillers[0])
                    except StopIteration:
                        fillers.popleft()

            def emit_qt(c, qin, QTt):
                # QT = Wq^T @ queryT + bq, one yield per completed j-chain
                for j in range(N_E):
                    ps = psA.tile([128, SC], F32, tag="acc", name=f"qacc{c}_{j}")
                    for e in range(N_E):
                        nc.tensor.matmul(
                            ps, lhsT=Wq_sb[e][:, j * 128:(j + 1) * 128], rhs=qin[e],
                            start=(e == 0), stop=(e == N_E - 1),
                        )
                    nc.vector.tensor_scalar_add(QTt[j], ps, bq_sb[:, j:j + 1])
                    yield

            def emit_outproj(c, OTt):
                # out = OT^T @ Wo + bo, one yield per completed chain
                for st in range(N_LT):
                    sts = slice(st * 128, (st + 1) * 128)
                    for half in range(2):
                        ps = psA.tile([128, SC], F32, tag="acc",
                                      name=f"oacc{c}_{st}_{half}")
                        for j in range(N_E):
                            nc.tensor.matmul(
                                ps, lhsT=OTt[j][:, sts],
                                rhs=Wo_sb[j][:, half * SC:(half + 1) * SC],
                                start=(j == 0), stop=(j == N_E - 1),
                            )
                        ob = stp.tile([128, SC], F32, tag="ob",
                                      name=f"ob{c}_{st}_{half}")
                        nc.vector.tensor_add(
                            ob, ps, bo_bc[:, half * SC:(half + 1) * SC])
                        nc.sync.dma_start(
                            out=out[c * SC + st * 128:c * SC + (st + 1) * 128,
                                    half * SC:(half + 1) * SC],
                            in_=ob,
                        )
                        yield

            QT_t = {0: [io.tile([128, SC], MMDT, tag=f"QT{j}", name=f"QT{j}_0")
                        for j in range(N_E)]}
            fillers.append(emit_qt(0, qin0, QT_t[0]))
            drain()

            OT_t = {}
            for c in range(N_SC):
                if c + 1 < N_SC:
                    qin = [io.tile([128, SC], MMDT, tag=f"qin{e}",
                                   name=f"qin{e}_{c + 1}") for e in range(N_E)]
                    for e in range(N_E):
                        nc.gpsimd.dma_start(
                            out=qin[e],
                            in_=queryT[e * 128:(e + 1) * 128,
                                       (c + 1) * SC:(c + 2) * SC])
                    QT_t[c + 1] = [io.tile([128, SC], MMDT, tag=f"QT{j}",
                                           name=f"QT{j}_{c + 1}")
                                   for j in range(N_E)]
                    fillers.append(emit_qt(c + 1, qin, QT_t[c + 1]))
                OT_t[c] = [io.tile([128, SC], MMDT, tag=f"OT{j}", name=f"OT{j}_{c}")
                           for j in range(N_E)]
                QTc = QT_t.pop(c)
                OTc = OT_t[c]

                pending = None  # deferred normalization of the previous hp
                for hp in range(HP):
                    att = [io.tile([128, 2 * SC], MMDT, tag=f"att{lt}",
                                   name=f"att{lt}_{c}_{hp}") for lt in range(N_LT)]
                    for lt in range(N_LT):
                        lts = slice(lt * 128, (lt + 1) * 128)
                        scp = psS.tile([128, 2 * SC], F32, tag="sc",
                                       name=f"sc{c}_{hp}_{lt}")
                        for u in range(2):
                            rows = slice(u * 64, (u + 1) * 64)
                            nc.tensor.matmul(
                                scp[:, u * SC:(u + 1) * SC],
                                lhsT=KT[hp][rows, lts], rhs=QTc[hp][rows, :],
                                start=True, stop=True,
                            )
                        nc.scalar.activation(
                            att[lt], scp, Exp,
                            bias=mb_sb[:, lt:lt + 1], scale=SCALE,
                        )
                        if lt == 1:
                            pump(1)
                        elif lt == 2 and pending is not None:
                            pending()
                            pending = None
                        elif lt == 3:
                            pump(1)
                    pvs = []
                    for u in range(2):
                        h = 2 * hp + u
                        pv = psV.tile([D + 1, SC], F32, tag="pv",
                                      name=f"pv{h}_{c}")
                        for lt in range(N_LT):
                            nc.tensor.matmul(
                                pv, lhsT=Vt[lt][:, h * (D + 1):(h + 1) * (D + 1)],
                                rhs=att[lt][:, u * SC:(u + 1) * SC],
                                start=(lt == 0), stop=(lt == N_LT - 1),
                            )
                        pvs.append(pv)
                    rc = stp.tile([1, 2 * SC], MMDT, tag="rc", name=f"rc{c}_{hp}")
                    with nc.allow_low_precision(reason="bf16 recip of denom"):
                        for u in range(2):
                            nc.vector.reciprocal(
                                rc[:, u * SC:(u + 1) * SC], pvs[u][D:D + 1, :])

                    def make_norm(hp, pvs, rc, OTd):
                        def norm():
                            # bcast 1/denom across 64 partitions per head via
                            # K=1 matmul, then scale the raw PV output
                            psB = psA.tile([128, SC], F32, tag="acc",
                                           name=f"bcp{c}_{hp}")
                            for u in range(2):
                                nc.tensor.matmul(
                                    psB[u * 64:(u + 1) * 64, :],
                                    lhsT=ones_r[:, 0:64],
                                    rhs=rc[:, u * SC:(u + 1) * SC],
                                    start=True, stop=True,
                                )
                            bc = stp.tile([64, 2 * SC], F32, tag="bc",
                                          name=f"bcs{c}_{hp}")
                            for u in range(2):
                                nc.vector.tensor_copy(
                                    bc[:, u * SC:(u + 1) * SC],
                                    psB[u * 64:(u + 1) * 64, :])
                            for u in range(2):
                                nc.vector.tensor_mul(
                                    OTd[u * 64:(u + 1) * 64, :],
                                    pvs[u][0:D, :],
                                    bc[:, u * SC:(u + 1) * SC])
                        return norm

                    pending = make_norm(hp, pvs, rc, OTc[hp])
                pending()

                fillers.append(emit_outproj(c, OTc))
            drain()

    if not nc.is_finalized():
        nc.finalize()
    return nc


def kernel(query, guide_vector, attention_mask, Wt, bt, Wq, bq, Wkv, bkv, Wo, bo):
    global _CACHED_NC
    query = np.asarray(query, dtype=np.float32)
    guide_vector = np.asarray(guide_vector, dtype=np.float32)
    attention_mask = np.asarray(attention_mask)
    Wt = np.asarray(Wt, dtype=np.float32)
    bt = np.asarray(bt, dtype=np.float32)
    bq = np.asarray(bq, dtype=np.float32)
    bkv = np.asarray(bkv, dtype=np.float32)
    bo = np.asarray(bo, dtype=np.float32)
    Wkv = np.asarray(Wkv, dtype=np.float32)

    # fold the text projection into the kv projection (host-side, fp32)
    Wf = Wt @ Wkv                       # [TE, 2E]
    bf = bt @ Wkv + bkv                 # [2E]
    Wk_m = np.ascontiguousarray(Wf[:, :E]).astype(NPDT)
    Wv_m = np.ascontiguousarray(Wf[:, E:]).astype(NPDT)
    Wq_m = np.asarray(Wq, dtype=np.float32).astype(NPDT)
    Wo_m = np.asarray(Wo, dtype=np.float32).astype(NPDT)
    bk_m = np.ascontiguousarray(bf[:E])
    bv_m = bf[E:].astype(NPDT)

    if _CACHED_NC is None:
        _CACHED_NC = build_nc()
    nc = _CACHED_NC

    mb = np.where(attention_mask == 0, np.float32(-1e9), np.float32(0.0))
    in_maps = []
    for b in range(B):
        in_maps.append({
            "queryT": np.ascontiguousarray(query[b].T).astype(NPDT),
            "guideT": np.ascontiguousarray(guide_vector[b].T).astype(NPDT),
            "Wq": Wq_m, "Wk": Wk_m, "Wv": Wv_m, "Wo": Wo_m,
            "bq": bq, "bk": bk_m, "bv_r": bv_m,
            "bo": bo, "mbias": mb[b].astype(np.float32),
        })
    res = run_bass_kernel_spmd(nc, in_maps, list(range(B)), trace=TRACE)
    if TRACE:
        kernel.last_exec_time_ns = res.exec_time_ns
        kernel.last_results = res
    return np.stack([res.results[b]["out"] for b in range(B)])


# revision 12
# speedup vs baseline: 1.6657x; 1.2174x over previous
"""Cross-attention Trainium2 kernel for nn_CrossAttention_37495064494692.

B=8 batches sharded 1/core across 8 NeuronCores (data parallel).
Per core: full cross-attention for one batch element in
feature-on-partitions ("transposed") layouts. Matmuls in bf16 with fp32
PSUM accumulation; softmax math fp32.

Host-side the text projection is folded into the kv projection:
  Wk = Wt @ Wkv[:, :E],  bk = bt @ Wkv[:, :E] + bkv[:E]   (same for v)
so the device computes
  KT  = Wk^T @ guideT + bk          [E, L]
  V   = guideT^T @ Wv + bv          [L, E]  stored per head with a ones
                                    column (v_aug) so the PV matmul also
                                    emits the softmax denominator row
  QT  = Wq^T @ queryT + bq          [E, S]  per 512-col s-chunk
  sT  = KT_h^T(l-tile) @ QT_h       [L, S]  two heads -> one 2-bank PSUM
                                    pair, one [128,1024] EXP each
  OTr_h = [v_h | 1]^T @ aT          [65, S] row 64 = denominator
  OT_h = OTr_h * bcast(1/denom)     bcast via K=1 matmul (no DRAM trip)
  out  = OT^T @ Wo + bo             [S, E]

The main loop software-pipelines across s-chunks: QT(c+1) and
out-projection(c-1) matmul chains are interleaved ("pumped") between the
score/PV bursts of chunk c so the PE never idles while the ACT engine
works through the EXPs.
"""
import sys

sys.path.insert(0, "/opt/trn_rl_repo")

from collections import deque

import ml_dtypes
import numpy as np

import concourse.bacc as bacc
import concourse.bass as bass
import concourse.tile as tile
from concourse import mybir
from concourse.bass_utils import run_bass_kernel_spmd

F32 = mybir.dt.float32
BF16 = mybir.dt.bfloat16
MMDT = BF16
NPDT = ml_dtypes.bfloat16

B, S, L = 8, 2048, 512
E, TE, H = 1024, 768, 16
D = E // H
SCALE = D ** -0.5

SC = 512              # s-chunk width
N_SC = S // SC        # 4 s-chunks
N_E = E // 128        # 8 E-chunks
N_TE = TE // 128      # 6 TE-chunks
N_LT = L // 128       # 4 L-tiles
HP = H // 2           # 8 head pairs

TRACE = False
_CACHED_NC = None


def build_nc():
    nc = bacc.Bacc()

    queryT = nc.declare_dram_parameter("queryT", [E, S], MMDT, isOutput=False)
    guideT = nc.declare_dram_parameter("guideT", [TE, L], MMDT, isOutput=False)
    Wq = nc.declare_dram_parameter("Wq", [E, E], MMDT, isOutput=False)
    Wk = nc.declare_dram_parameter("Wk", [TE, E], MMDT, isOutput=False)
    Wv = nc.declare_dram_parameter("Wv", [TE, E], MMDT, isOutput=False)
    Wo = nc.declare_dram_parameter("Wo", [E, E], MMDT, isOutput=False)
    bq = nc.declare_dram_parameter("bq", [E], F32, isOutput=False)
    bk = nc.declare_dram_parameter("bk", [E], F32, isOutput=False)
    bv_r = nc.declare_dram_parameter("bv_r", [E], MMDT, isOutput=False)
    bo = nc.declare_dram_parameter("bo", [E], F32, isOutput=False)
    mbias = nc.declare_dram_parameter("mbias", [L], F32, isOutput=False)
    selm = nc.declare_dram_parameter("selm", [16, H * 128 // 2], MMDT,
                                     isOutput=False)
    out = nc.declare_dram_parameter("out", [S, E], F32, isOutput=True)

    Exp = mybir.ActivationFunctionType.Exp

    with tile.TileContext(nc) as tc:
        with (
            tc.tile_pool(name="res", bufs=1) as res,
            tc.tile_pool(name="io", bufs=2) as io,
            tc.tile_pool(name="stp", bufs=3) as stp,
            tc.tile_pool(name="psA", bufs=2, space="PSUM") as psA,
            tc.tile_pool(name="psS", bufs=2, space="PSUM") as psS,
            tc.tile_pool(name="psV", bufs=2, space="PSUM") as psV,
        ):
            # ---- resident small tensors ----
            bq_sb = res.tile([128, N_E], F32, tag="bq")
            bk_sb = res.tile([128, N_E], F32, tag="bk")
            mb_sb = res.tile([128, N_LT], F32, tag="mb")
            nc.sync.dma_start(out=bq_sb, in_=bq.rearrange("(t p) -> p t", p=128))
            nc.sync.dma_start(out=bk_sb, in_=bk.rearrange("(t p) -> p t", p=128))
            nc.sync.dma_start(out=mb_sb, in_=mbias.rearrange("(t p) -> p t", p=128))
            bo_bc = res.tile([128, E], F32, tag="bo")
            bo_ap = bo[:]
            nc.gpsimd.dma_start(
                out=bo_bc,
                in_=bass.AP(tensor=bo_ap.tensor, offset=bo_ap.offset,
                            ap=[[0, 128], [1, E]]),
            )
            bv_row = res.tile([1, E], MMDT, tag="bvr")
            nc.sync.dma_start(out=bv_row, in_=bv_r.rearrange("(one f) -> one f", one=1))
            ones_f = res.tile([1, 128], F32, tag="ones_f")
            ones_r = res.tile([1, 128], MMDT, tag="ones_r")
            nc.vector.memset(ones_f, 1.0)
            nc.scalar.copy(ones_r, ones_f)
            onesc_f = res.tile([128, H], F32, tag="onesc")
            nc.vector.memset(onesc_f, 1.0)
            # per-head-pair broadcast selector: SEL[h, hp*128+m] == 1 iff
            # head h == 2*hp + (m // 64); bcast matmul SEL_hp^T @ recips
            # replicates head h's 1/denom row onto its 64 OT partitions
            SEL = res.tile([16, H * 128 // 2], MMDT, tag="SEL")
            nc.sync.dma_start(out=SEL, in_=selm[:, :])
            # denominator staging: head h parks at partition 32*(h//4),
            # col slot (h%4)*SC (DVE writes need 32-aligned start partitions)
            dn_st = res.tile([97, 4 * SC], F32, tag="dnst")

            # ---- long-lived activations + resident weights ----
            KT = [res.tile([128, L], MMDT, tag=f"KT{j}", name=f"KT{j}")
                  for j in range(N_E)]
            # v_aug: head h cols h*65..h*65+64, col h*65+64 == 1.0
            Vt = [res.tile([128, H * (D + 1)], MMDT, tag=f"V{lt}", name=f"V{lt}")
                  for lt in range(N_LT)]
            Wq_sb = [res.tile([128, E], MMDT, tag=f"wq{e}", name=f"wq{e}")
                     for e in range(N_E)]
            Wo_sb = [res.tile([128, E], MMDT, tag=f"wo{e}", name=f"wo{e}")
                     for e in range(N_E)]

            # chunk-0 query + weights prefetch on the gpsimd DMA queue
            qin0 = [io.tile([128, SC], MMDT, tag=f"qin{e}", name=f"qin{e}_0")
                    for e in range(N_E)]
            for e in range(N_E):
                nc.gpsimd.dma_start(out=qin0[e], in_=queryT[e * 128:(e + 1) * 128, 0:SC])
            for e in range(N_E):
                nc.gpsimd.dma_start(out=Wq_sb[e], in_=Wq[e * 128:(e + 1) * 128, :])
            for e in range(N_E):
                nc.gpsimd.dma_start(out=Wo_sb[e], in_=Wo[e * 128:(e + 1) * 128, :])

            # ================= prologue: KT and V =================
            with tc.tile_pool(name="pro", bufs=1) as pro:
                g_in = [pro.tile([128, L], MMDT, tag=f"gin{t}", name=f"gin{t}")
                        for t in range(N_TE)]
                Wk_sb = [pro.tile([128, E], MMDT, tag=f"wk{t}", name=f"wk{t}")
                         for t in range(N_TE)]
                Wv_sb = [pro.tile([128, E], MMDT, tag=f"wv{t}", name=f"wv{t}")
                         for t in range(N_TE)]
                for t in range(N_TE):
                    nc.sync.dma_start(out=g_in[t], in_=guideT[t * 128:(t + 1) * 128, :])
                for t in range(N_TE):
                    nc.sync.dma_start(out=Wk_sb[t], in_=Wk[t * 128:(t + 1) * 128, :])
                for t in range(N_TE):
                    nc.sync.dma_start(out=Wv_sb[t], in_=Wv[t * 128:(t + 1) * 128, :])

                # KT = Wk^T @ guideT + bk
                for j in range(N_E):
                    ps = psA.tile([128, SC], F32, tag="acc", name=f"kacc{j}")
                    for t in range(N_TE):
                        nc.tensor.matmul(
                            ps, lhsT=Wk_sb[t][:, j * 128:(j + 1) * 128], rhs=g_in[t],
                            start=(t == 0), stop=(t == N_TE - 1),
                        )
                    nc.vector.tensor_scalar_add(KT[j], ps, bk_sb[:, j:j + 1])

                # V = guideT^T @ Wv + bv, into v_aug layout
                for lt in range(N_LT):
                    va = Vt[lt].rearrange("p (h c) -> p h c", c=D + 1)
                    for half in range(2):
                        ps = psA.tile([128, SC], F32, tag="acc", name=f"vacc{lt}_{half}")
                        for t in range(N_TE):
                            nc.tensor.matmul(
                                ps, lhsT=g_in[t][:, lt * 128:(lt + 1) * 128],
                                rhs=Wv_sb[t][:, half * SC:(half + 1) * SC],
                                start=(t == 0), stop=False,
                            )
                        nc.tensor.matmul(
                            ps, lhsT=ones_r,
                            rhs=bv_row[:, half * SC:(half + 1) * SC],
                            start=False, stop=True,
                        )
                        nc.vector.tensor_copy(
                            va[:, half * 8:(half + 1) * 8, 0:D],
                            ps.rearrange("p (h c) -> p h c", c=D),
                        )
                    nc.vector.tensor_copy(
                        va[:, :, D:D + 1],
                        onesc_f.rearrange("p (h c) -> p h c", c=1),
                    )

            # ================= main loop =================
            fillers = deque()

            def pump(n):
                while n > 0 and fillers:
                    try:
                        next(fillers[0])
                        n -= 1
                    except StopIteration:
                        fillers.popleft()

            def drain():
                while fillers:
                    try:
                        next(fillers[0])
                    except StopIteration:
                        fillers.popleft()

            def emit_qt(c, qin, QTt):
                # QT = Wq^T @ queryT + bq, one yield per completed j-chain
                for j in range(N_E):
                    ps = psA.tile([128, SC], F32, tag="acc", name=f"qacc{c}_{j}")
                    for e in range(N_E):
                        nc.tensor.matmul(
                            ps, lhsT=Wq_sb[e][:, j * 128:(j + 1) * 128], rhs=qin[e],
                            start=(e == 0), stop=(e == N_E - 1),
                        )
                    nc.vector.tensor_scalar_add(QTt[j], ps, bq_sb[:, j:j + 1])
                    yield

            def emit_outproj(c, OTt):
                # out = OT^T @ Wo + bo, one yield per completed chain
                for st in range(N_LT):
                    sts = slice(st * 128, (st + 1) * 128)
                    for half in range(2):
                        ps = psA.tile([128, SC], F32, tag="acc",
                                      name=f"oacc{c}_{st}_{half}")
                        for j in range(N_E):
                            nc.tensor.matmul(
                                ps, lhsT=OTt[j][:, sts],
                                rhs=Wo_sb[j][:, half * SC:(half + 1) * SC],
                                start=(j == 0), stop=(j == N_E - 1),
                            )
                        ob = stp.tile([128, SC], F32, tag="ob",
                                      name=f"ob{c}_{st}_{half}")
                        nc.vector.tensor_add(
                            ob, ps, bo_bc[:, half * SC:(half + 1) * SC])
                        nc.sync.dma_start(
                            out=out[c * SC + st * 128:c * SC + (st + 1) * 128,
                                    half * SC:(half + 1) * SC],
                            in_=ob,
                        )
                        yield

            def emit_norm(c, OTrw, OTt, rca):
                # per head pair: replicate 1/denom onto the 64 partitions of
                # each head (K=16 selector matmul), then scale the raw PV
                # output: OT = OTraw * bcast   (SBUF bf16 * PSUM f32)
                for hp in range(HP):
                    psB = psA.tile([128, SC], F32, tag="acc",
                                   name=f"bcp{c}_{hp}")
                    nc.tensor.matmul(
                        psB, lhsT=SEL[:, hp * 128:(hp + 1) * 128], rhs=rca,
                        start=True, stop=True,
                    )
                    nc.vector.tensor_mul(OTt[hp], OTrw[hp], psB)
                    yield

            QT_t = {0: [io.tile([128, SC], MMDT, tag=f"QT{j}", name=f"QT{j}_0")
                        for j in range(N_E)]}
            fillers.append(emit_qt(0, qin0, QT_t[0]))
            drain()

            for c in range(N_SC):
                if c + 1 < N_SC:
                    qin = [io.tile([128, SC], MMDT, tag=f"qin{e}",
                                   name=f"qin{e}_{c + 1}") for e in range(N_E)]
                    for e in range(N_E):
                        nc.gpsimd.dma_start(
                            out=qin[e],
                            in_=queryT[e * 128:(e + 1) * 128,
                                       (c + 1) * SC:(c + 2) * SC])
                    QT_t[c + 1] = [io.tile([128, SC], MMDT, tag=f"QT{j}",
                                           name=f"QT{j}_{c + 1}")
                                   for j in range(N_E)]
                    fillers.append(emit_qt(c + 1, qin, QT_t[c + 1]))
                OTraw = [io.tile([128, SC], MMDT, tag=f"OTr{j}",
                                 name=f"OTr{j}_{c}") for j in range(N_E)]
                OTc = [io.tile([128, SC], MMDT, tag=f"OT{j}", name=f"OT{j}_{c}")
                       for j in range(N_E)]
                QTc = QT_t.pop(c)

                for hp in range(HP):
                    att = [io.tile([128, 2 * SC], MMDT, tag=f"att{lt}",
                                   name=f"att{lt}_{c}_{hp}") for lt in range(N_LT)]
                    for lt in range(N_LT):
                        lts = slice(lt * 128, (lt + 1) * 128)
                        scp = psS.tile([128, 2 * SC], F32, tag="sc",
                                       name=f"sc{c}_{hp}_{lt}")
                        for u in range(2):
                            rows = slice(u * 64, (u + 1) * 64)
                            nc.tensor.matmul(
                                scp[:, u * SC:(u + 1) * SC],
                                lhsT=KT[hp][rows, lts], rhs=QTc[hp][rows, :],
                                start=True, stop=True,
                            )
                        nc.scalar.activation(
                            att[lt], scp, Exp,
                            bias=mb_sb[:, lt:lt + 1], scale=SCALE,
                        )
                        if lt >= 1:
                            pump(1)
                    for u in range(2):
                        h = 2 * hp + u
                        pv = psV.tile([D + 1, SC], F32, tag="pv",
                                      name=f"pv{h}_{c}")
                        for lt in range(N_LT):
                            nc.tensor.matmul(
                                pv, lhsT=Vt[lt][:, h * (D + 1):(h + 1) * (D + 1)],
                                rhs=att[lt][:, u * SC:(u + 1) * SC],
                                start=(lt == 0), stop=(lt == N_LT - 1),
                            )
                        nc.vector.tensor_copy(
                            OTraw[hp][u * 64:(u + 1) * 64, :], pv[0:D, :])
                        nc.vector.tensor_copy(
                            dn_st[32 * (h // 4):32 * (h // 4) + 1,
                                  (h % 4) * SC:(h % 4 + 1) * SC],
                            pv[D:D + 1, :])

                # gather the 16 staged denom rows onto 16 partitions with one
                # strided DMA, then a single batched reciprocal
                dn_c = stp.tile([H, SC], F32, tag="dna", name=f"dn_{c}")
                nc.sync.dma_start(
                    out=dn_c,
                    in_=dn_st[0:97:32, :].rearrange("p (s c) -> p s c", c=SC),
                )
                rca = stp.tile([H, SC], MMDT, tag="rca", name=f"rca{c}")
                with nc.allow_low_precision(reason="bf16 recip of denom"):
                    nc.vector.reciprocal(rca, dn_c)
                fillers.append(emit_norm(c, OTraw, OTc, rca))
                fillers.append(emit_outproj(c, OTc))
            drain()

    if not nc.is_finalized():
        nc.finalize()
    return nc


def kernel(query, guide_vector, attention_mask, Wt, bt, Wq, bq, Wkv, bkv, Wo, bo):
    global _CACHED_NC
    query = np.asarray(query, dtype=np.float32)
    guide_vector = np.asarray(guide_vector, dtype=np.float32)
    attention_mask = np.asarray(attention_mask)
    Wt = np.asarray(Wt, dtype=np.float32)
    bt = np.asarray(bt, dtype=np.float32)
    bq = np.asarray(bq, dtype=np.float32)
    bkv = np.asarray(bkv, dtype=np.float32)
    bo = np.asarray(bo, dtype=np.float32)
    Wkv = np.asarray(Wkv, dtype=np.float32)

    # fold the text projection into the kv projection (host-side, fp32)
    Wf = Wt @ Wkv                       # [TE, 2E]
    bf = bt @ Wkv + bkv                 # [2E]
    Wk_m = np.ascontiguousarray(Wf[:, :E]).astype(NPDT)
    Wv_m = np.ascontiguousarray(Wf[:, E:]).astype(NPDT)
    Wq_m = np.asarray(Wq, dtype=np.float32).astype(NPDT)
    Wo_m = np.asarray(Wo, dtype=np.float32).astype(NPDT)
    bk_m = np.ascontiguousarray(bf[:E])
    bv_m = bf[E:].astype(NPDT)

    if _CACHED_NC is None:
        _CACHED_NC = build_nc()
    nc = _CACHED_NC

    selm = np.zeros((16, H * 128 // 2), dtype=NPDT)
    for h in range(H):
        col = (h // 2) * 128 + (h % 2) * 64
        selm[h, col:col + 64] = 1.0

    mb = np.where(attention_mask == 0, np.float32(-1e9), np.float32(0.0))
    in_maps = []
    for b in range(B):
        in_maps.append({
            "queryT": np.ascontiguousarray(query[b].T).astype(NPDT),
            "guideT": np.ascontiguousarray(guide_vector[b].T).astype(NPDT),
            "Wq": Wq_m, "Wk": Wk_m, "Wv": Wv_m, "Wo": Wo_m,
            "bq": bq, "bk": bk_m, "bv_r": bv_m,
            "bo": bo, "mbias": mb[b].astype(np.float32), "selm": selm,
        })
    res = run_bass_kernel_spmd(nc, in_maps, list(range(B)), trace=TRACE)
    if TRACE:
        kernel.last_exec_time_ns = res.exec_time_ns
        kernel.last_results = res
    return np.stack([res.results[b]["out"] for b in range(B)])


# revision 17
# speedup vs baseline: 2.0531x; 1.2326x over previous
"""Cross-attention Trainium2 kernel for nn_CrossAttention_37495064494692.

B=8 batches sharded 1/core across 8 NeuronCores (data parallel).
Per core: full cross-attention for one batch element in
feature-on-partitions ("transposed") layouts. Matmuls in bf16 with fp32
PSUM accumulation; softmax math fp32.

Host-side the text projection is folded into the kv projection:
  Wk = Wt @ Wkv[:, :E],  bk = bt @ Wkv[:, :E] + bkv[:E]   (same for v)
so the device computes
  KT  = Wk^T @ guideT + bk          [E, L]
  V   = guideT^T @ Wv + bv          [L, E]  stored per head with a ones
                                    column (v_aug) so the PV matmul also
                                    emits the softmax denominator row
  QT  = Wq^T @ queryT + bq          [E, S]  per 512-col s-chunk
  sT  = KT_h^T(l-tile) @ QT_h       [L, S]  two heads -> one 2-bank PSUM
                                    pair, one [128,1024] EXP each
  OTr_h = [v_h | 1]^T @ aT          [65, S] row 64 = denominator
  OT_h = OTr_h * bcast(1/denom)     bcast via K=1 matmul (no DRAM trip)
  out  = OT^T @ Wo + bo             [S, E]

The main loop software-pipelines across s-chunks: QT(c+1) and
out-projection(c-1) matmul chains are interleaved ("pumped") between the
score/PV bursts of chunk c so the PE never idles while the ACT engine
works through the EXPs.
"""
import sys

sys.path.insert(0, "/opt/trn_rl_repo")

from collections import deque

import ml_dtypes
import numpy as np

import concourse.bacc as bacc
import concourse.bass as bass
import concourse.tile as tile
from concourse import mybir
from concourse.bass_utils import run_bass_kernel_spmd

F32 = mybir.dt.float32
BF16 = mybir.dt.bfloat16
MMDT = BF16
NPDT = ml_dtypes.bfloat16

B, S, L = 8, 2048, 512
E, TE, H = 1024, 768, 16
D = E // H
SCALE = D ** -0.5

SC = 512              # s-chunk width
N_SC = S // SC        # 4 s-chunks
N_E = E // 128        # 8 E-chunks
N_TE = TE // 128      # 6 TE-chunks
N_LT = L // 128       # 4 L-tiles
HP = H // 2           # 8 head pairs

TRACE = False
_CACHED_NC = None


def build_nc():
    nc = bacc.Bacc()

    queryT = nc.declare_dram_parameter("queryT", [E, S], MMDT, isOutput=False)
    guideT = nc.declare_dram_parameter("guideT", [TE, L], MMDT, isOutput=False)
    Wq = nc.declare_dram_parameter("Wq", [E, E], MMDT, isOutput=False)
    Wk = nc.declare_dram_parameter("Wk", [TE, E], MMDT, isOutput=False)
    Wv = nc.declare_dram_parameter("Wv", [TE, E], MMDT, isOutput=False)
    Wo = nc.declare_dram_parameter("Wo", [E, E], MMDT, isOutput=False)
    bq = nc.declare_dram_parameter("bq", [E], F32, isOutput=False)
    bk = nc.declare_dram_parameter("bk", [E], F32, isOutput=False)
    bv_r = nc.declare_dram_parameter("bv_r", [E], MMDT, isOutput=False)
    bo = nc.declare_dram_parameter("bo", [E], F32, isOutput=False)
    mbias = nc.declare_dram_parameter("mbias", [L], F32, isOutput=False)
    selm = nc.declare_dram_parameter("selm", [16, H * 128 // 2], MMDT,
                                     isOutput=False)
    out = nc.declare_dram_parameter("out", [S, E], F32, isOutput=True)

    Exp = mybir.ActivationFunctionType.Exp

    with tile.TileContext(nc) as tc:
        with (
            tc.tile_pool(name="res", bufs=1) as res,
            tc.tile_pool(name="io", bufs=2) as io,
            tc.tile_pool(name="stp", bufs=3) as stp,
            tc.tile_pool(name="psA", bufs=2, space="PSUM") as psA,
            tc.tile_pool(name="psS", bufs=2, space="PSUM") as psS,
            tc.tile_pool(name="psV", bufs=2, space="PSUM") as psV,
        ):
            # ---- resident small tensors ----
            bq_sb = res.tile([128, N_E], F32, tag="bq")
            bk_sb = res.tile([128, N_E], F32, tag="bk")
            mb_sb = res.tile([128, N_LT], F32, tag="mb")
            nc.sync.dma_start(out=bq_sb, in_=bq.rearrange("(t p) -> p t", p=128))
            nc.sync.dma_start(out=bk_sb, in_=bk.rearrange("(t p) -> p t", p=128))
            nc.sync.dma_start(out=mb_sb, in_=mbias.rearrange("(t p) -> p t", p=128))
            bo_bc = res.tile([128, E], F32, tag="bo")
            bo_ap = bo[:]
            nc.gpsimd.dma_start(
                out=bo_bc,
                in_=bass.AP(tensor=bo_ap.tensor, offset=bo_ap.offset,
                            ap=[[0, 128], [1, E]]),
            )
            bv_row = res.tile([1, E], MMDT, tag="bvr")
            nc.sync.dma_start(out=bv_row, in_=bv_r.rearrange("(one f) -> one f", one=1))
            ones_f = res.tile([1, 128], F32, tag="ones_f")
            ones_r = res.tile([1, 128], MMDT, tag="ones_r")
            nc.vector.memset(ones_f, 1.0)
            nc.scalar.copy(ones_r, ones_f)
            onesc_f = res.tile([128, H], F32, tag="onesc")
            nc.vector.memset(onesc_f, 1.0)
            # per-head-pair broadcast selector: SEL[h, hp*128+m] == 1 iff
            # head h == 2*hp + (m // 64); bcast matmul SEL_hp^T @ recips
            # replicates head h's 1/denom row onto its 64 OT partitions
            SEL = res.tile([16, H * 128 // 2], MMDT, tag="SEL")
            nc.sync.dma_start(out=SEL, in_=selm[:, :])
            # denominator staging: head h parks at partition 32*(h//4),
            # col slot (h%4)*SC (DVE writes need 32-aligned start partitions)
            dn_st = res.tile([97, 4 * SC], MMDT, tag="dnst")

            # ---- long-lived activations + resident weights ----
            KT = [res.tile([128, L], MMDT, tag=f"KT{j}", name=f"KT{j}")
                  for j in range(N_E)]
            # v_aug: head h cols h*65..h*65+64, col h*65+64 == 1.0
            Vt = [res.tile([128, H * (D + 1)], MMDT, tag=f"V{lt}", name=f"V{lt}")
                  for lt in range(N_LT)]
            Wq_sb = [res.tile([128, E], MMDT, tag=f"wq{e}", name=f"wq{e}")
                     for e in range(N_E)]
            Wo_sb = [res.tile([128, E], MMDT, tag=f"wo{e}", name=f"wo{e}")
                     for e in range(N_E)]

            # chunk-0 query + weights prefetch on the gpsimd DMA queue
            qin0 = [io.tile([128, SC], MMDT, tag=f"qin{e}", name=f"qin{e}_0")
                    for e in range(N_E)]
            for e in range(N_E):
                nc.gpsimd.dma_start(out=qin0[e], in_=queryT[e * 128:(e + 1) * 128, 0:SC])
            for e in range(N_E):
                nc.gpsimd.dma_start(out=Wq_sb[e], in_=Wq[e * 128:(e + 1) * 128, :])
            for e in range(N_E):
                nc.gpsimd.dma_start(out=Wo_sb[e], in_=Wo[e * 128:(e + 1) * 128, :])

            # ================= main loop =================
            fillers = deque()

            def pump(n):
                while n > 0 and fillers:
                    try:
                        next(fillers[0])
                        n -= 1
                    except StopIteration:
                        fillers.popleft()

            def drain():
                while fillers:
                    try:
                        next(fillers[0])
                    except StopIteration:
                        fillers.popleft()

            def emit_qt(c, qin, QTt):
                # QT = Wq^T @ queryT + bq, one yield per completed j-chain
                for j in range(N_E):
                    ps = psA.tile([128, SC], F32, tag="acc", name=f"qacc{c}_{j}")
                    for e in range(N_E):
                        nc.tensor.matmul(
                            ps, lhsT=Wq_sb[e][:, j * 128:(j + 1) * 128], rhs=qin[e],
                            start=(e == 0), stop=(e == N_E - 1),
                        )
                    nc.vector.tensor_scalar_add(QTt[j], ps, bq_sb[:, j:j + 1])
                    yield

            def emit_outproj(c, OTt):
                # out = OT^T @ Wo + bo, one yield per completed chain
                for st in range(N_LT):
                    sts = slice(st * 128, (st + 1) * 128)
                    for half in range(2):
                        ps = psA.tile([128, SC], F32, tag="acc",
                                      name=f"oacc{c}_{st}_{half}")
                        for j in range(N_E):
                            nc.tensor.matmul(
                                ps, lhsT=OTt[j][:, sts],
                                rhs=Wo_sb[j][:, half * SC:(half + 1) * SC],
                                start=(j == 0), stop=(j == N_E - 1),
                            )
                        ob = stp.tile([128, SC], F32, tag="ob",
                                      name=f"ob{c}_{st}_{half}")
                        nc.vector.tensor_add(
                            ob, ps, bo_bc[:, half * SC:(half + 1) * SC])
                        nc.sync.dma_start(
                            out=out[c * SC + st * 128:c * SC + (st + 1) * 128,
                                    half * SC:(half + 1) * SC],
                            in_=ob,
                        )
                        yield

            def emit_norm(c, OTrw, OTt, rca):
                # per head pair: replicate 1/denom onto the 64 partitions of
                # each head (K=16 selector matmul), then scale the raw PV
                # output: OT = OTraw * bcast   (SBUF bf16 * PSUM f32)
                for hp in range(HP):
                    psB = psA.tile([128, SC], F32, tag="acc",
                                   name=f"bcp{c}_{hp}")
                    nc.tensor.matmul(
                        psB, lhsT=SEL[:, hp * 128:(hp + 1) * 128], rhs=rca,
                        start=True, stop=True,
                    )
                    nc.vector.tensor_mul(OTt[hp], OTrw[hp], psB)
                    yield

            def emit_scores(c, hp, QTc, atts):
                att = [io.tile([128, 2 * SC], MMDT, tag=f"att{lt}",
                               name=f"att{lt}_{c}_{hp}", bufs=4)
                       for lt in range(N_LT)]
                for lt in range(N_LT):
                    lts = slice(lt * 128, (lt + 1) * 128)
                    scp = psS.tile([128, 2 * SC], F32, tag="sc",
                                   name=f"sc{c}_{hp}_{lt}")
                    for u in range(2):
                        rows = slice(u * 64, (u + 1) * 64)
                        nc.tensor.matmul(
                            scp[:, u * SC:(u + 1) * SC],
                            lhsT=KT[hp][rows, lts], rhs=QTc[hp][rows, :],
                            start=True, stop=True,
                        )
                    nc.scalar.activation(
                        att[lt], scp, Exp,
                        bias=mb_sb[:, lt:lt + 1], scale=SCALE,
                    )
                    if lt >= 1:
                        pump(1)
                atts[hp] = att

            def emit_pv(c, hp, atts, OTraw):
                att = atts.pop(hp)
                for u in range(2):
                    h = 2 * hp + u
                    pv = psV.tile([D + 1, SC], F32, tag="pv", name=f"pv{h}_{c}")
                    for lt in range(N_LT):
                        nc.tensor.matmul(
                            pv, lhsT=Vt[lt][:, h * (D + 1):(h + 1) * (D + 1)],
                            rhs=att[lt][:, u * SC:(u + 1) * SC],
                            start=(lt == 0), stop=(lt == N_LT - 1),
                        )
                    nc.vector.tensor_copy(
                        OTraw[hp][u * 64:(u + 1) * 64, :], pv[0:D, :])
                    nc.vector.tensor_copy(
                        dn_st[32 * (h // 4):32 * (h // 4) + 1,
                              (h % 4) * SC:(h % 4 + 1) * SC],
                        pv[D:D + 1, :])

            # ========== prologue: KT, V, QT(0), early chunk-0 scores ======
            QT_t = {0: [io.tile([128, SC], MMDT, tag=f"QT{j}", name=f"QT{j}_0")
                        for j in range(N_E)]}
            atts0 = {}
            with tc.tile_pool(name="pro", bufs=1) as pro:
                g_in = [pro.tile([128, L], MMDT, tag=f"gin{t}", name=f"gin{t}")
                        for t in range(N_TE)]
                Wk_sb = [pro.tile([128, E], MMDT, tag=f"wk{t}", name=f"wk{t}")
                         for t in range(N_TE)]
                Wv_sb = [pro.tile([128, E], MMDT, tag=f"wv{t}", name=f"wv{t}")
                         for t in range(N_TE)]
                for t in range(N_TE):
                    nc.sync.dma_start(out=g_in[t], in_=guideT[t * 128:(t + 1) * 128, :])
                for t in range(N_TE):
                    nc.sync.dma_start(out=Wk_sb[t], in_=Wk[t * 128:(t + 1) * 128, :])
                for t in range(N_TE):
                    nc.sync.dma_start(out=Wv_sb[t], in_=Wv[t * 128:(t + 1) * 128, :])

                def kt_chain(j):
                    # KT = Wk^T @ guideT + bk
                    ps = psA.tile([128, SC], F32, tag="acc", name=f"kacc{j}")
                    for t in range(N_TE):
                        nc.tensor.matmul(
                            ps, lhsT=Wk_sb[t][:, j * 128:(j + 1) * 128], rhs=g_in[t],
                            start=(t == 0), stop=(t == N_TE - 1),
                        )
                    nc.vector.tensor_scalar_add(KT[j], ps, bk_sb[:, j:j + 1])

                # interleave KT and QT(0) chains 1:1, pre-issuing the first
                # three chunk-0 score/EXP blocks so ACT starts early
                qt0_gen = emit_qt(0, qin0, QT_t[0])
                kt_chain(0); next(qt0_gen)
                kt_chain(1); next(qt0_gen)
                emit_scores(0, 0, QT_t[0], atts0)
                kt_chain(2); next(qt0_gen)
                emit_scores(0, 1, QT_t[0], atts0)
                kt_chain(3); next(qt0_gen)
                emit_scores(0, 2, QT_t[0], atts0)
                for j in range(4, N_E):
                    kt_chain(j)
                    next(qt0_gen, None)
                for _ in qt0_gen:
                    pass

                # V = guideT^T @ Wv + bv, into v_aug layout
                for lt in range(N_LT):
                    va = Vt[lt].rearrange("p (h c) -> p h c", c=D + 1)
                    for half in range(2):
                        ps = psA.tile([128, SC], F32, tag="acc", name=f"vacc{lt}_{half}")
                        for t in range(N_TE):
                            nc.tensor.matmul(
                                ps, lhsT=g_in[t][:, lt * 128:(lt + 1) * 128],
                                rhs=Wv_sb[t][:, half * SC:(half + 1) * SC],
                                start=(t == 0), stop=False,
                            )
                        nc.tensor.matmul(
                            ps, lhsT=ones_r,
                            rhs=bv_row[:, half * SC:(half + 1) * SC],
                            start=False, stop=True,
                        )
                        nc.vector.tensor_copy(
                            va[:, half * 8:(half + 1) * 8, 0:D],
                            ps.rearrange("p (h c) -> p h c", c=D),
                        )
                    nc.vector.tensor_copy(
                        va[:, :, D:D + 1],
                        onesc_f.rearrange("p (h c) -> p h c", c=1),
                    )

            for c in range(N_SC):
                if c + 1 < N_SC:
                    qin = [io.tile([128, SC], MMDT, tag=f"qin{e}",
                                   name=f"qin{e}_{c + 1}") for e in range(N_E)]
                    for e in range(N_E):
                        nc.gpsimd.dma_start(
                            out=qin[e],
                            in_=queryT[e * 128:(e + 1) * 128,
                                       (c + 1) * SC:(c + 2) * SC])
                    QT_t[c + 1] = [io.tile([128, SC], MMDT, tag=f"QT{j}",
                                           name=f"QT{j}_{c + 1}")
                                   for j in range(N_E)]
                    fillers.append(emit_qt(c + 1, qin, QT_t[c + 1]))
                OTraw = [io.tile([128, SC], MMDT, tag=f"OTr{j}",
                                 name=f"OTr{j}_{c}") for j in range(N_E)]
                OTc = [io.tile([128, SC], MMDT, tag=f"OT{j}", name=f"OT{j}_{c}")
                       for j in range(N_E)]
                QTc = QT_t.pop(c)

                # PV lags scores by `lag` head pairs so ACT always has runway
                atts = atts0 if c == 0 else {}
                lag = 3 if c == 0 else 1
                for hp in range(3 if c == 0 else 0, HP):
                    emit_scores(c, hp, QTc, atts)
                    if hp - lag >= 0:
                        emit_pv(c, hp - lag, atts, OTraw)
                for hp in range(HP - lag, HP):
                    emit_pv(c, hp, atts, OTraw)

                # gather the 16 staged denom rows onto 16 partitions with one
                # strided DMA, then a single batched reciprocal
                dn_c = stp.tile([H, SC], MMDT, tag="dna", name=f"dn_{c}")
                nc.sync.dma_start(
                    out=dn_c,
                    in_=dn_st[0:97:32, :].rearrange("p (s c) -> p s c", c=SC),
                )
                rca = stp.tile([H, SC], MMDT, tag="rca", name=f"rca{c}")
                with nc.allow_low_precision(reason="bf16 recip of denom"):
                    nc.vector.reciprocal(rca, dn_c)
                fillers.append(emit_norm(c, OTraw, OTc, rca))
                fillers.append(emit_outproj(c, OTc))
            drain()

    if not nc.is_finalized():
        nc.finalize()
    return nc


def kernel(query, guide_vector, attention_mask, Wt, bt, Wq, bq, Wkv, bkv, Wo, bo):
    global _CACHED_NC
    query = np.asarray(query, dtype=np.float32)
    guide_vector = np.asarray(guide_vector, dtype=np.float32)
    attention_mask = np.asarray(attention_mask)
    Wt = np.asarray(Wt, dtype=np.float32)
    bt = np.asarray(bt, dtype=np.float32)
    bq = np.asarray(bq, dtype=np.float32)
    bkv = np.asarray(bkv, dtype=np.float32)
    bo = np.asarray(bo, dtype=np.float32)
    Wkv = np.asarray(Wkv, dtype=np.float32)

    # fold the text projection into the kv projection (host-side, fp32)
    Wf = Wt @ Wkv                       # [TE, 2E]
    bf = bt @ Wkv + bkv                 # [2E]
    Wk_m = np.ascontiguousarray(Wf[:, :E]).astype(NPDT)
    Wv_m = np.ascontiguousarray(Wf[:, E:]).astype(NPDT)
    Wq_m = np.asarray(Wq, dtype=np.float32).astype(NPDT)
    Wo_m = np.asarray(Wo, dtype=np.float32).astype(NPDT)
    bk_m = np.ascontiguousarray(bf[:E])
    bv_m = bf[E:].astype(NPDT)

    if _CACHED_NC is None:
        _CACHED_NC = build_nc()
    nc = _CACHED_NC

    selm = np.zeros((16, H * 128 // 2), dtype=NPDT)
    for h in range(H):
        col = (h // 2) * 128 + (h % 2) * 64
        selm[h, col:col + 64] = 1.0

    mb = np.where(attention_mask == 0, np.float32(-1e9), np.float32(0.0))
    in_maps = []
    for b in range(B):
        in_maps.append({
            "queryT": np.ascontiguousarray(query[b].T).astype(NPDT),
            "guideT": np.ascontiguousarray(guide_vector[b].T).astype(NPDT),
            "Wq": Wq_m, "Wk": Wk_m, "Wv": Wv_m, "Wo": Wo_m,
            "bq": bq, "bk": bk_m, "bv_r": bv_m,
            "bo": bo, "mbias": mb[b].astype(np.float32), "selm": selm,
        })
    res = run_bass_kernel_spmd(nc, in_maps, list(range(B)), trace=TRACE)
    if TRACE:
        kernel.last_exec_time_ns = res.exec_time_ns
        kernel.last_results = res
    return np.stack([res.results[b]["out"] for b in range(B)])


# revision 23
# speedup vs baseline: 2.0793x; 1.0128x over previous
"""Cross-attention Trainium2 kernel for nn_CrossAttention_37495064494692.

B=8 batches sharded 1/core across 8 NeuronCores (data parallel).
Per core: full cross-attention for one batch element in
feature-on-partitions ("transposed") layouts. Matmuls in bf16 with fp32
PSUM accumulation; softmax math fp32.

Host-side the text projection is folded into the kv projection:
  Wk = Wt @ Wkv[:, :E],  bk = bt @ Wkv[:, :E] + bkv[:E]   (same for v)
so the device computes
  KT  = Wk^T @ guideT + bk          [E, L]
  V   = guideT^T @ Wv + bv          [L, E]  stored per head with a ones
                                    column (v_aug) so the PV matmul also
                                    emits the softmax denominator row
  QT  = Wq^T @ queryT + bq          [E, S]  per 512-col s-chunk
  sT  = KT_h^T(l-tile) @ QT_h       [L, S]  two heads -> one 2-bank PSUM
                                    pair, one [128,1024] EXP each
  OTr_h = [v_h | 1]^T @ aT          [65, S] row 64 = denominator
  OT_h = OTr_h * bcast(1/denom)     bcast via K=1 matmul (no DRAM trip)
  out  = OT^T @ Wo + bo             [S, E]

The main loop software-pipelines across s-chunks: QT(c+1) and
out-projection(c-1) matmul chains are interleaved ("pumped") between the
score/PV bursts of chunk c so the PE never idles while the ACT engine
works through the EXPs.
"""
import sys

sys.path.insert(0, "/opt/trn_rl_repo")

from collections import deque

import ml_dtypes
import numpy as np

import concourse.bacc as bacc
import concourse.bass as bass
import concourse.tile as tile
from concourse import mybir
from concourse.bass_utils import run_bass_kernel_spmd

F32 = mybir.dt.float32
BF16 = mybir.dt.bfloat16
MMDT = BF16
NPDT = ml_dtypes.bfloat16

B, S, L = 8, 2048, 512
E, TE, H = 1024, 768, 16
D = E // H
SCALE = D ** -0.5

SC = 512              # s-chunk width
N_SC = S // SC        # 4 s-chunks
N_E = E // 128        # 8 E-chunks
N_TE = TE // 128      # 6 TE-chunks
N_LT = L // 128       # 4 L-tiles
HP = H // 2           # 8 head pairs

TRACE = False
_CACHED_NC = None


def build_nc():
    nc = bacc.Bacc()

    queryT = nc.declare_dram_parameter("queryT", [E, S], MMDT, isOutput=False)
    guideT = nc.declare_dram_parameter("guideT", [TE, L], MMDT, isOutput=False)
    Wq = nc.declare_dram_parameter("Wq", [E, E], MMDT, isOutput=False)
    Wk = nc.declare_dram_parameter("Wk", [TE, E], MMDT, isOutput=False)
    Wv = nc.declare_dram_parameter("Wv", [TE, E], MMDT, isOutput=False)
    Wo = nc.declare_dram_parameter("Wo", [E, E], MMDT, isOutput=False)
    bq = nc.declare_dram_parameter("bq", [E], F32, isOutput=False)
    bk = nc.declare_dram_parameter("bk", [E], F32, isOutput=False)
    bv_r = nc.declare_dram_parameter("bv_r", [E], MMDT, isOutput=False)
    bo = nc.declare_dram_parameter("bo", [E], F32, isOutput=False)
    mbias = nc.declare_dram_parameter("mbias", [L], F32, isOutput=False)
    selm = nc.declare_dram_parameter("selm", [16, H * 128 // 2], MMDT,
                                     isOutput=False)
    out = nc.declare_dram_parameter("out", [S, E], F32, isOutput=True)

    Exp = mybir.ActivationFunctionType.Exp

    with tile.TileContext(nc) as tc:
        with (
            tc.tile_pool(name="res", bufs=1) as res,
            tc.tile_pool(name="io", bufs=2) as io,
            tc.tile_pool(name="stp", bufs=3) as stp,
            tc.tile_pool(name="psA", bufs=2, space="PSUM") as psA,
            tc.tile_pool(name="psS", bufs=2, space="PSUM") as psS,
            tc.tile_pool(name="psV", bufs=2, space="PSUM") as psV,
        ):
            # ---- resident small tensors ----
            bq_sb = res.tile([128, N_E], F32, tag="bq")
            bk_sb = res.tile([128, N_E], F32, tag="bk")
            mb_sb = res.tile([128, N_LT], F32, tag="mb")
            nc.sync.dma_start(out=bq_sb, in_=bq.rearrange("(t p) -> p t", p=128))
            nc.sync.dma_start(out=bk_sb, in_=bk.rearrange("(t p) -> p t", p=128))
            nc.sync.dma_start(out=mb_sb, in_=mbias.rearrange("(t p) -> p t", p=128))
            bv_row = res.tile([1, E], MMDT, tag="bvr")
            nc.sync.dma_start(out=bv_row, in_=bv_r.rearrange("(one f) -> one f", one=1))
            ones_f = res.tile([1, 128], F32, tag="ones_f")
            ones_r = res.tile([1, 128], MMDT, tag="ones_r")
            nc.vector.memset(ones_f, 1.0)
            nc.scalar.copy(ones_r, ones_f)
            onesc_f = res.tile([128, H], F32, tag="onesc")
            nc.vector.memset(onesc_f, 1.0)
            # per-head-pair broadcast selector: SEL[h, hp*128+m] == 1 iff
            # head h == 2*hp + (m // 64); bcast matmul SEL_hp^T @ recips
            # replicates head h's 1/denom row onto its 64 OT partitions
            SEL = res.tile([16, H * 128 // 2], MMDT, tag="SEL")
            nc.sync.dma_start(out=SEL, in_=selm[:, :])
            # denominator staging: head h parks at partition 32*(h//4),
            # col slot (h%4)*SC (DVE writes need 32-aligned start partitions)
            dn_st = res.tile([97, 4 * SC], MMDT, tag="dnst")

            # ---- long-lived activations + resident weights ----
            KT = [res.tile([128, L], MMDT, tag=f"KT{j}", name=f"KT{j}")
                  for j in range(N_E)]
            # v_aug: head h cols h*65..h*65+64, col h*65+64 == 1.0
            Vt = [res.tile([128, H * (D + 1)], MMDT, tag=f"V{lt}", name=f"V{lt}")
                  for lt in range(N_LT)]
            Wq_sb = [res.tile([128, E], MMDT, tag=f"wq{e}", name=f"wq{e}")
                     for e in range(N_E)]
            Wo_sb = [res.tile([128, E], MMDT, tag=f"wo{e}", name=f"wo{e}")
                     for e in range(N_E)]

            # chunk-0 query + weights prefetch; Wq is split across both DMA
            # queues so the QT(0) chains can start early
            qin0 = [io.tile([128, SC], MMDT, tag=f"qin{e}", name=f"qin{e}_0")
                    for e in range(N_E)]
            for e in range(N_E):
                nc.gpsimd.dma_start(out=qin0[e], in_=queryT[e * 128:(e + 1) * 128, 0:SC])
            for e in range(4, N_E):
                nc.gpsimd.dma_start(out=Wq_sb[e], in_=Wq[e * 128:(e + 1) * 128, :])
            for e in range(N_E):
                nc.gpsimd.dma_start(out=Wo_sb[e], in_=Wo[e * 128:(e + 1) * 128, :])
            bo_bc = res.tile([128, E], F32, tag="bo")
            bo_ap = bo[:]
            nc.gpsimd.dma_start(
                out=bo_bc,
                in_=bass.AP(tensor=bo_ap.tensor, offset=bo_ap.offset,
                            ap=[[0, 128], [1, E]]),
            )

            # ================= main loop =================
            fillers = deque()

            def pump(n):
                while n > 0 and fillers:
                    try:
                        next(fillers[0])
                        n -= 1
                    except StopIteration:
                        fillers.popleft()

            def drain():
                while fillers:
                    try:
                        next(fillers[0])
                    except StopIteration:
                        fillers.popleft()

            def emit_qt(c, qin, QTt):
                # QT = Wq^T @ queryT + bq, one yield per completed j-chain
                for j in range(N_E):
                    ps = psA.tile([128, SC], F32, tag="acc", name=f"qacc{c}_{j}")
                    for e in range(N_E):
                        nc.tensor.matmul(
                            ps, lhsT=Wq_sb[e][:, j * 128:(j + 1) * 128], rhs=qin[e],
                            start=(e == 0), stop=(e == N_E - 1),
                        )
                    nc.vector.tensor_scalar_add(QTt[j], ps, bq_sb[:, j:j + 1])
                    yield

            def emit_outproj(c, OTt):
                # out = OT^T @ Wo + bo, one yield per completed chain
                for st in range(N_LT):
                    sts = slice(st * 128, (st + 1) * 128)
                    for half in range(2):
                        ps = psA.tile([128, SC], F32, tag="acc",
                                      name=f"oacc{c}_{st}_{half}")
                        for j in range(N_E):
                            nc.tensor.matmul(
                                ps, lhsT=OTt[j][:, sts],
                                rhs=Wo_sb[j][:, half * SC:(half + 1) * SC],
                                start=(j == 0), stop=(j == N_E - 1),
                            )
                        ob = stp.tile([128, SC], F32, tag="ob",
                                      name=f"ob{c}_{st}_{half}")
                        nc.vector.tensor_add(
                            ob, ps, bo_bc[:, half * SC:(half + 1) * SC])
                        nc.sync.dma_start(
                            out=out[c * SC + st * 128:c * SC + (st + 1) * 128,
                                    half * SC:(half + 1) * SC],
                            in_=ob,
                        )
                        yield

            def emit_norm(c, OTrw, OTt, rca):
                # per head pair: replicate 1/denom onto the 64 partitions of
                # each head (K=16 selector matmul), then scale the raw PV
                # output: OT = OTraw * bcast   (SBUF bf16 * PSUM f32)
                for hp in range(HP):
                    psB = psA.tile([128, SC], F32, tag="acc",
                                   name=f"bcp{c}_{hp}")
                    nc.tensor.matmul(
                        psB, lhsT=SEL[:, hp * 128:(hp + 1) * 128], rhs=rca,
                        start=True, stop=True,
                    )
                    nc.vector.tensor_mul(OTt[hp], OTrw[hp], psB)
                    yield

            def emit_scores(c, hp, QTc, atts):
                att = [io.tile([128, 2 * SC], MMDT, tag=f"att{lt}",
                               name=f"att{lt}_{c}_{hp}", bufs=4)
                       for lt in range(N_LT)]
                for lt in range(N_LT):
                    lts = slice(lt * 128, (lt + 1) * 128)
                    scp = psS.tile([128, 2 * SC], F32, tag="sc",
                                   name=f"sc{c}_{hp}_{lt}")
                    for u in range(2):
                        rows = slice(u * 64, (u + 1) * 64)
                        nc.tensor.matmul(
                            scp[:, u * SC:(u + 1) * SC],
                            lhsT=KT[hp][rows, lts], rhs=QTc[hp][rows, :],
                            start=True, stop=True,
                        )
                    nc.scalar.activation(
                        att[lt], scp, Exp,
                        bias=mb_sb[:, lt:lt + 1], scale=SCALE,
                    )
                    if lt >= 1:
                        pump(1)
                atts[hp] = att

            def emit_pv(c, hp, atts, OTraw):
                att = atts.pop(hp)
                for u in range(2):
                    h = 2 * hp + u
                    pv = psV.tile([D + 1, SC], F32, tag="pv", name=f"pv{h}_{c}")
                    for lt in range(N_LT):
                        nc.tensor.matmul(
                            pv, lhsT=Vt[lt][:, h * (D + 1):(h + 1) * (D + 1)],
                            rhs=att[lt][:, u * SC:(u + 1) * SC],
                            start=(lt == 0), stop=(lt == N_LT - 1),
                        )
                    nc.vector.tensor_copy(
                        OTraw[hp][u * 64:(u + 1) * 64, :], pv[0:D, :])
                    nc.vector.tensor_copy(
                        dn_st[32 * (h // 4):32 * (h // 4) + 1,
                              (h % 4) * SC:(h % 4 + 1) * SC],
                        pv[D:D + 1, :])

            # ========== prologue: KT, V, QT(0), early chunk-0 scores ======
            QT_t = {0: [io.tile([128, SC], MMDT, tag=f"QT{j}", name=f"QT{j}_0")
                        for j in range(N_E)]}
            atts0 = {}
            with tc.tile_pool(name="pro", bufs=1) as pro:
                g_in = [pro.tile([128, L], MMDT, tag=f"gin{t}", name=f"gin{t}")
                        for t in range(N_TE)]
                Wk_sb = [pro.tile([128, E], MMDT, tag=f"wk{t}", name=f"wk{t}")
                         for t in range(N_TE)]
                Wv_sb = [pro.tile([128, E], MMDT, tag=f"wv{t}", name=f"wv{t}")
                         for t in range(N_TE)]
                for t in range(N_TE):
                    nc.sync.dma_start(out=g_in[t], in_=guideT[t * 128:(t + 1) * 128, :])
                    nc.sync.dma_start(out=Wk_sb[t], in_=Wk[t * 128:(t + 1) * 128, :])
                for e in range(4):
                    nc.sync.dma_start(out=Wq_sb[e], in_=Wq[e * 128:(e + 1) * 128, :])
                for t in range(N_TE):
                    nc.sync.dma_start(out=Wv_sb[t], in_=Wv[t * 128:(t + 1) * 128, :])

                def kt_chain(j):
                    # KT = Wk^T @ guideT + bk
                    ps = psA.tile([128, SC], F32, tag="acc", name=f"kacc{j}")
                    for t in range(N_TE):
                        nc.tensor.matmul(
                            ps, lhsT=Wk_sb[t][:, j * 128:(j + 1) * 128], rhs=g_in[t],
                            start=(t == 0), stop=(t == N_TE - 1),
                        )
                    nc.vector.tensor_scalar_add(KT[j], ps, bk_sb[:, j:j + 1])

                # interleave KT and QT(0) chains 1:1, pre-issuing the first
                # three chunk-0 score/EXP blocks so ACT starts early
                qt0_gen = emit_qt(0, qin0, QT_t[0])
                kt_chain(0); next(qt0_gen)
                kt_chain(1); next(qt0_gen)
                emit_scores(0, 0, QT_t[0], atts0)
                kt_chain(2); next(qt0_gen)
                emit_scores(0, 1, QT_t[0], atts0)
                kt_chain(3); next(qt0_gen)
                emit_scores(0, 2, QT_t[0], atts0)
                for j in range(4, N_E):
                    kt_chain(j)
                    next(qt0_gen, None)
                for _ in qt0_gen:
                    pass

                # V = guideT^T @ Wv + bv, into v_aug layout
                for lt in range(N_LT):
                    va = Vt[lt].rearrange("p (h c) -> p h c", c=D + 1)
                    for half in range(2):
                        ps = psA.tile([128, SC], F32, tag="acc", name=f"vacc{lt}_{half}")
                        for t in range(N_TE):
                            nc.tensor.matmul(
                                ps, lhsT=g_in[t][:, lt * 128:(lt + 1) * 128],
                                rhs=Wv_sb[t][:, half * SC:(half + 1) * SC],
                                start=(t == 0), stop=False,
                            )
                        nc.tensor.matmul(
                            ps, lhsT=ones_r,
                            rhs=bv_row[:, half * SC:(half + 1) * SC],
                            start=False, stop=True,
                        )
                        nc.vector.tensor_copy(
                            va[:, half * 8:(half + 1) * 8, 0:D],
                            ps.rearrange("p (h c) -> p h c", c=D),
                        )
                    nc.vector.tensor_copy(
                        va[:, :, D:D + 1],
                        onesc_f.rearrange("p (h c) -> p h c", c=1),
                    )

            def make_finalize(c, OTraw, OTc):
                def fin():
                    # gather the 16 staged denom rows onto 16 partitions with
                    # one strided DMA, then a single batched reciprocal
                    dn_c = stp.tile([H, SC], MMDT, tag="dna", name=f"dn_{c}")
                    nc.sync.dma_start(
                        out=dn_c,
                        in_=dn_st[0:97:32, :].rearrange("p (s c) -> p s c", c=SC),
                    )
                    rca = stp.tile([H, SC], MMDT, tag="rca", name=f"rca{c}")
                    with nc.allow_low_precision(reason="bf16 recip of denom"):
                        nc.vector.reciprocal(rca, dn_c)
                    fillers.append(emit_norm(c, OTraw, OTc, rca))
                    fillers.append(emit_outproj(c, OTc))
                return fin

            # The last two PV chains of each chunk (and that chunk's
            # finalize) carry over into the next chunk's first score slots so
            # the score/EXP stream never pauses at a chunk boundary.
            carry = []
            for c in range(N_SC):
                if c + 1 < N_SC:
                    qin = [io.tile([128, SC], MMDT, tag=f"qin{e}",
                                   name=f"qin{e}_{c + 1}") for e in range(N_E)]
                    for e in range(N_E):
                        nc.gpsimd.dma_start(
                            out=qin[e],
                            in_=queryT[e * 128:(e + 1) * 128,
                                       (c + 1) * SC:(c + 2) * SC])
                    QT_t[c + 1] = [io.tile([128, SC], MMDT, tag=f"QT{j}",
                                           name=f"QT{j}_{c + 1}")
                                   for j in range(N_E)]
                    fillers.append(emit_qt(c + 1, qin, QT_t[c + 1]))
                OTraw = [io.tile([128, SC], MMDT, tag=f"OTr{j}",
                                 name=f"OTr{j}_{c}") for j in range(N_E)]
                OTc = [io.tile([128, SC], MMDT, tag=f"OT{j}", name=f"OT{j}_{c}")
                       for j in range(N_E)]
                QTc = QT_t.pop(c)

                atts = atts0 if c == 0 else {}
                n_own = HP if c == N_SC - 1 else HP - 2
                fin_c = make_finalize(c, OTraw, OTc)
                pv_queue = carry + [(c, hp, atts, OTraw,
                                     fin_c if hp == HP - 1 else None)
                                    for hp in range(n_own)]
                carry = [(c, hp, atts, OTraw, fin_c if hp == HP - 1 else None)
                         for hp in range(n_own, HP)]

                for hp in range(3 if c == 0 else 0, HP):
                    emit_scores(c, hp, QTc, atts)
                    if pv_queue:
                        pc, php, patts, pOTraw, pfin = pv_queue.pop(0)
                        emit_pv(pc, php, patts, pOTraw)
                        if pfin is not None:
                            pfin()
                while pv_queue:
                    pc, php, patts, pOTraw, pfin = pv_queue.pop(0)
                    emit_pv(pc, php, patts, pOTraw)
                    if pfin is not None:
                        pfin()
            drain()

    if not nc.is_finalized():
        nc.finalize()
    return nc


def kernel(query, guide_vector, attention_mask, Wt, bt, Wq, bq, Wkv, bkv, Wo, bo):
    global _CACHED_NC
    query = np.asarray(query, dtype=np.float32)
    guide_vector = np.asarray(guide_vector, dtype=np.float32)
    attention_mask = np.asarray(attention_mask)
    Wt = np.asarray(Wt, dtype=np.float32)
    bt = np.asarray(bt, dtype=np.float32)
    bq = np.asarray(bq, dtype=np.float32)
    bkv = np.asarray(bkv, dtype=np.float32)
    bo = np.asarray(bo, dtype=np.float32)
    Wkv = np.asarray(Wkv, dtype=np.float32)

    # fold the text projection into the kv projection (host-side, fp32)
    Wf = Wt @ Wkv                       # [TE, 2E]
    bf = bt @ Wkv + bkv                 # [2E]
    Wk_m = np.ascontiguousarray(Wf[:, :E]).astype(NPDT)
    Wv_m = np.ascontiguousarray(Wf[:, E:]).astype(NPDT)
    Wq_m = np.asarray(Wq, dtype=np.float32).astype(NPDT)
    Wo_m = np.asarray(Wo, dtype=np.float32).astype(NPDT)
    bk_m = np.ascontiguousarray(bf[:E])
    bv_m = bf[E:].astype(NPDT)

    if _CACHED_NC is None:
        _CACHED_NC = build_nc()
    nc = _CACHED_NC

    selm = np.zeros((16, H * 128 // 2), dtype=NPDT)
    for h in range(H):
        col = (h // 2) * 128 + (h % 2) * 64
        selm[h, col:col + 64] = 1.0

    mb = np.where(attention_mask == 0, np.float32(-1e9), np.float32(0.0))
    in_maps = []
    for b in range(B):
        in_maps.append({
            "queryT": np.ascontiguousarray(query[b].T).astype(NPDT),
            "guideT": np.ascontiguousarray(guide_vector[b].T).astype(NPDT),
            "Wq": Wq_m, "Wk": Wk_m, "Wv": Wv_m, "Wo": Wo_m,
            "bq": bq, "bk": bk_m, "bv_r": bv_m,
            "bo": bo, "mbias": mb[b].astype(np.float32), "selm": selm,
        })
    res = run_bass_kernel_spmd(nc, in_maps, list(range(B)), trace=TRACE)
    if TRACE:
        kernel.last_exec_time_ns = res.exec_time_ns
        kernel.last_results = res
    return np.stack([res.results[b]["out"] for b in range(B)])
